# revision 1
# baseline (speedup 1.0000x reference)
"""Trainium2 Bass kernel for nn_AttentionDist (attention + per-class logsumexp).

Math (per batch b):
    logits = queries @ context.T * mask          [Q, K]
    p      = softmax(logits, axis=-1)
    cv     = p @ context                          [Q, D]
    lsc[q,c] = logsumexp_{k: labels[k]==c} logp[q,k] = ln(S_c[q] / T[q])
  where P = exp(logits)  (no max-subtraction needed: |logits| <~ 50 fits f32),
        S_c = sum_{k in class c} P[q,k],  T = sum_k P[q,k].

Kernel strategy (data-parallel, core i <- batch i):
  MM1 (f32r):  logitsT[kt]  = cT[:,kt].T @ qT          -> PSUM [128k, 1024q]
  ACT:         P^T[kt]      = exp(logitsT[kt])         -> SBUF bf16
  MM2 (bf16):  acc         += [ctx|1|onehot][kt].T @ P^T[kt]  -> PSUM [86, 1024]
               rows 0:64 = cv.T unnorm, 64 = T, 65:86 = S.T
  Finale: PE-transpose acc to [q, 86] tiles, divide by T, ln for lsc.

The mask is folded in on the CPU by scaling cT columns (exact:
logits_k = (mask_k * c_k) . q), matching the reference's logits * mask.
"""

import numpy as np
import ml_dtypes

B, Q, K, D = 8, 1024, 4096, 64
C1 = 21
KT = K // 128  # 32 k-tiles
QT = Q // 128  # 8 q-tiles
W = D + 1 + C1  # 86 combo columns: [context | ones | onehot]

_CACHE = {}


def _build_graph():
    import concourse.bacc as bacc
    import concourse.tile as tile
    from concourse import mybir
    from concourse.masks import make_identity

    f32 = mybir.dt.float32
    f32r = mybir.dt.float32r
    bf16 = mybir.dt.bfloat16
    AF = mybir.ActivationFunctionType

    nc = bacc.Bacc()
    qT_d = nc.declare_dram_parameter("qT", [D, Q], f32r, isOutput=False)
    cT_d = nc.declare_dram_parameter("cT", [D, K], f32r, isOutput=False)
    ctxb_d = nc.declare_dram_parameter("ctxb", [K, D], bf16, isOutput=False)
    lab_d = nc.declare_dram_parameter("lab", [128, KT], f32, isOutput=False)
    cv_d = nc.declare_dram_parameter("cv", [Q, D], f32, isOutput=True)
    lsc_d = nc.declare_dram_parameter("lsc", [Q, C1], f32, isOutput=True)

    with tile.TileContext(nc) as tc:
        with (
            tc.tile_pool(name="singles", bufs=1) as singles,
            tc.tile_pool(name="p_pool", bufs=4) as p_pool,
            tc.tile_pool(name="combo_pool", bufs=4) as combo_pool,
            tc.tile_pool(name="fin_pool", bufs=2) as fin_pool,
            tc.tile_pool(name="small_pool", bufs=2) as small_pool,
            tc.tile_pool(name="mm1_ps", bufs=2, space="PSUM") as mm1_ps,
            tc.tile_pool(name="acc_ps", bufs=1, space="PSUM") as acc_ps,
            tc.tile_pool(name="fin_ps", bufs=2, space="PSUM") as fin_ps,
        ):
            ident = singles.tile([128, 128], f32)
            make_identity(nc, ident)
            iota = singles.tile([128, C1], f32)
            nc.gpsimd.iota(
                iota[:], pattern=[[1, C1]], base=0, channel_multiplier=0,
                allow_small_or_imprecise_dtypes=True,
            )
            qT_sb = singles.tile([D, Q], f32r)
            nc.sync.dma_start(qT_sb[:], qT_d[:])
            cT_sb = singles.tile([D, K], f32r)
            nc.sync.dma_start(cT_sb[:], cT_d[:])
            lab_sb = singles.tile([128, KT], f32)
            nc.sync.dma_start(lab_sb[:], lab_d[:])

            acc = acc_ps.tile([W, Q], f32)

            for kt in range(KT):
                ks = slice(kt * 128, (kt + 1) * 128)
                mm1 = mm1_ps.tile([128, Q], f32)
                for h in range(2):
                    qs = slice(h * 512, (h + 1) * 512)
                    nc.tensor.matmul(
                        mm1[:, qs], cT_sb[:, ks], qT_sb[:, qs],
                        start=True, stop=True,
                    )
                pT = p_pool.tile([128, Q], bf16)
                nc.scalar.activation(pT[:], mm1[:], AF.Exp)

                combo = combo_pool.tile([128, W], bf16)
                nc.sync.dma_start(combo[:, 0:D], ctxb_d[ks, :])
                nc.vector.memset(combo[:, D:D + 1], 1.0)
                nc.vector.tensor_scalar(
                    combo[:, D + 1:W], iota[:], lab_sb[:, kt:kt + 1], None,
                    op0=mybir.AluOpType.is_equal,
                )
                for h in range(2):
                    qs = slice(h * 512, (h + 1) * 512)
                    nc.tensor.matmul(
                        acc[:, qs], combo[:], pT[:, qs],
                        start=(kt == 0), stop=(kt == KT - 1),
                    )

            acc_sb = singles.tile([W, Q], f32)
            nc.vector.tensor_copy(acc_sb[:], acc[:])

            for qt in range(QT):
                qs = slice(qt * 128, (qt + 1) * 128)
                tp = fin_ps.tile([128, W], f32)
                nc.tensor.transpose(tp[:], acc_sb[:, qs], ident[0:W, 0:W])
                rec = small_pool.tile([128, 1], f32)
                nc.vector.reciprocal(rec[:], tp[:, D:D + 1])
                fin = fin_pool.tile([128, W], f32)
                nc.vector.tensor_scalar_mul(fin[:, 0:D], tp[:, 0:D], rec[:, 0:1])
                nc.vector.tensor_scalar_mul(
                    fin[:, D + 1:W], tp[:, D + 1:W], rec[:, 0:1]
                )
                nc.scalar.activation(fin[:, D + 1:W], fin[:, D + 1:W], AF.Ln)
                nc.sync.dma_start(cv_d[qs, :], fin[:, 0:D])
                nc.sync.dma_start(lsc_d[qs, :], fin[:, D + 1:W])

    nc.finalize()
    return nc


def _get_graph():
    if "nc" not in _CACHE:
        _CACHE["nc"] = _build_graph()
    return _CACHE["nc"]


def kernel(queries, context, context_labels, mask, num_classes,
           _profile=False):
    from concourse.bass_utils import run_bass_kernel_spmd

    queries = np.asarray(queries, dtype=np.float32)
    context = np.asarray(context, dtype=np.float32)
    labels = np.asarray(context_labels)
    mask = np.asarray(mask, dtype=np.float32)
    assert queries.shape == (B, Q, D) and context.shape == (B, K, D)
    assert int(num_classes) + 1 == C1

    nc = _get_graph()
    in_maps = []
    for b in range(B):
        cT = context[b].T * mask[b][None, :]  # fold mask into MM1 operand
        in_maps.append({
            "qT": np.ascontiguousarray(queries[b].T),
            "cT": np.ascontiguousarray(cT, dtype=np.float32),
            "ctxb": np.asarray(context[b], dtype=ml_dtypes.bfloat16),
            "lab": np.ascontiguousarray(
                labels[b].reshape(KT, 128).T
            ).astype(np.float32),
        })

    res = run_bass_kernel_spmd(
        nc, in_maps, list(range(B)), trace=bool(_profile)
    )
    kernel._last_result = res
    cv = np.stack([res.results[i]["cv"] for i in range(B)])
    lsc = np.stack([res.results[i]["lsc"] for i in range(B)])
    return cv, lsc
